# revision 34
# baseline (speedup 1.0000x reference)
"""BFP activation quantization kernel for Trainium2 (8 NeuronCores).

Problem: x (64, 256, 56, 56) fp32. Channels grouped in blocks of 32; each
block shares the max frexp-exponent emax; mantissas truncated to
`mantissa_bits` bits relative to 2^emax:
    q = trunc(x / 2^(emax-mb)) * 2^(emax-mb)

Math (bit-exact):
  - emax: Pw = bits(max_block |x|) & 0x7F800000 = 2^(emax-1) (bitcast);
    R = reciprocal(Pw) = 2^(1-emax) exact (power of two).
  - ya' = x * R (exact, |ya'| < 2). ACT folds the remaining 2^(mb-1)
    factor into its per-instruction scale: with ya = x * 2^(mb-emax),
      t16 = rne16(2^(mb-1)*ya' - 0.5),  u16 = rne16(2^(mb-1)*ya' + 0.5)
    (the fp32->int16 converter rounds to nearest-even; ya +- 0.5 exact).
  - Signed trunc without an abs/sign-restore pass, exact for every case
    including rne ties and both zeros:
      2*trunc(ya) = (t16 + u16 + [x<0]) & ~1
    with [x<0] = (t16 >> 15) & 1 (t16 < 0 iff ya < 0).
  - q = (2*trunc(ya)) * 2^(emax-mb-1); int16 -> f32 convert and the
    power-of-two multiply are exact. SC2 = 2^(emax-mb-1) in bf16 (exact).

Engine split per tile (1 image; instruction cost model numbers):
  DVE  (24.2us): reduce |x| over block (strided); Pt; R; SC2; ya';
                 V = t16+u16; g = (t16>>15)&1; VW = V+g; TR = VW & ~1.
  ACT  (10.9us + store issue): t16; u16; store DMAs (lagged 2 tiles so
                 the pool wait never blocks compute dispatch).
  Pool (12.5us): Q = TR * bc(SC2)   (i16 x bf16 broadcast -> f32).
  SP:  load DMAs only (issued eagerly; never blocked behind stores).
DMA fabric floor is ~17.8us/tile (51.4 MB/core at ~360 GB/s).

Pool/ACT offload notes (probed on HW): Pool tensor_tensor supports
mult with f32 out (incl. broadcast + mixed i16/bf16 in) but NOT
max/min/abs_max or i16 out; tensor_scalar/scalar_tensor_tensor do not
exist on Pool; AluOpType.mod doesn't codegen anywhere. DVE/ACT/Pool
have no same-engine RAW interlocks -> every cross-instruction
dependency gets a standalone wait_ge on a semaphore counter; all
counter values are precomputed so engine programs can be emitted in
any order.

Layout (per core, n-shard of 8 images): tile = 1 image; SBUF partition
p = (b<8, sigma<16), free = (c<32, s<196), s contiguous in DRAM (784 B
runs).

Sharding: data-parallel on N across 8 cores, no cross-core communication.
"""

import numpy as np

N_CORES = 8
N, C, H, W = 64, 256, 56, 56
HW = H * W                   # 3136
N_PER_CORE = N // N_CORES    # 8
B = 8                        # channel blocks
SIG = 16                     # spatial chunks per image
C_IN = 32                    # channels per block
S = HW // SIG                # 196
P = B * SIG                  # 128 partitions
TILES = N_PER_CORE           # 8 (one image per tile)
DMAS = B                     # dma_starts per tile per direction
INC = 16 * DMAS              # load-sem increment per tile (128)

TRACE = False
LAST_RESULTS = None
_CACHE = {}


def _counters():
    """Precompute per-engine semaphore counter values."""
    # DVE: order f0 f1 b0 f2 b1 ... f7 b6 b7; front=5 insts, back=4.
    red, rc, frontd, vd, vwd, trd = {}, {}, {}, {}, {}, {}
    k = 0
    order = [("f", 0), ("f", 1)]
    for t in range(2, TILES):
        order += [("b", t - 2), ("f", t)]
    order += [("b", TILES - 2), ("b", TILES - 1)]
    for kind, t in order:
        if kind == "f":
            red[t] = k + 1
            rc[t] = k + 3    # R (reciprocal) done
            k += 5
            frontd[t] = k
        else:
            vd[t] = k + 1
            vwd[t] = k + 3
            k += 4
            trd[t] = k

    # ACT: per t: t16(t), u16(t). (Store DMAs don't inc act_sem.)
    t16c = {t: 2 * t + 1 for t in range(TILES)}
    u16c = {t: 2 * t + 2 for t in range(TILES)}

    # Pool: YAh2(0); per t>=1: YAh2(t), Q(t-1); then Q(TILES-1).
    yac, pq = {0: 1}, {}
    p = 1
    for t in range(1, TILES):
        yac[t] = p = p + 1
        pq[t - 1] = p = p + 1
    pq[TILES - 1] = p = p + 1

    return red, rc, frontd, vd, vwd, trd, t16c, u16c, yac, pq


def _build(mbits: int):
    import concourse.bass as bass
    from concourse import mybir

    nc = bass.Bass()
    x_in = nc.declare_dram_parameter(
        "x", [N_PER_CORE, C, HW], mybir.dt.float32, isOutput=False
    )
    q_out = nc.declare_dram_parameter(
        "q", [N_PER_CORE, C, HW], mybir.dt.float32, isOutput=True
    )
    src = x_in[:].rearrange("n (b c) (g s) -> n b g c s", c=C_IN, s=S)
    dst = q_out[:].rearrange("n (b c) (g s) -> n b g c s", c=C_IN, s=S)

    i32, f32, i16, bf16 = (
        mybir.dt.int32, mybir.dt.float32, mybir.dt.int16, mybir.dt.bfloat16
    )
    Alu = mybir.AluOpType

    red, rc, frontd, vd, vwd, trd, t16c, u16c, yac, pq = _counters()

    from contextlib import ExitStack
    es = ExitStack()
    with es:
        sb = lambda nm, shape, dt: es.enter_context(nc.sbuf_tensor(nm, shape, dt))
        X0 = sb("X0", [P, C_IN, S], f32); X1 = sb("X1", [P, C_IN, S], f32)
        YA0 = sb("YA0", [P, C_IN, S], f32); YA1 = sb("YA1", [P, C_IN, S], f32)
        Q0 = sb("Q0", [P, C_IN, S], f32); Q1 = sb("Q1", [P, C_IN, S], f32)
        T0 = sb("T0", [P, C_IN, S], i16); T1 = sb("T1", [P, C_IN, S], i16)
        U0 = sb("U0", [P, C_IN, S], i16); U1 = sb("U1", [P, C_IN, S], i16)
        M = sb("Mt", [P, S], f32); Pt = sb("Ptt", [P, S], i32)
        R = sb("Rt", [P, S], f32)
        SCS = [sb(f"SC{i}", [P, S], bf16) for i in range(4)]
        load_sem = es.enter_context(nc.semaphore())
        store_sem = es.enter_context(nc.semaphore())
        dve_sem = es.enter_context(nc.semaphore())
        act_sem = es.enter_context(nc.semaphore())
        pool_sem = es.enter_context(nc.semaphore())
        block = es.enter_context(nc.Block())
        X = [X0, X1]
        YA = [YA0, YA1]
        QO = [Q0, Q1]
        T16 = [T0, T1]
        U16 = [U0, U1]
        SC = SCS
        HC = C_IN // 2  # ya' split: DVE does c<16, Pool does c>=16

        def bch(ap):
            return ap.unsqueeze(1).broadcast_to((P, HC, S))

        def bc(ap):
            return ap.unsqueeze(1).broadcast_to((P, C_IN, S))

        @block.vector
        def _(vector):
            k = 0

            def step(inst):
                nonlocal k
                inst.then_inc(dve_sem, 1)
                k += 1
                vector.wait_ge(dve_sem, k)

            def front(t):
                nonlocal k
                # M = max_block |x| (strided free-dim reduce)
                vector.wait_ge(load_sem, INC * (t + 1))
                step(vector.tensor_reduce(
                    out=M[:], in_=X[t % 2][:].rearrange("p c s -> p s c"),
                    axis=mybir.AxisListType.X, op=Alu.max,
                    apply_absolute_value=True,
                ))
                assert red[t] == k
                step(vector.tensor_scalar(
                    out=Pt[:], in0=M[:].bitcast(i32),
                    scalar1=0x7F800000, scalar2=None, op0=Alu.bitwise_and,
                ))
                # R free once Pool ya'-high(t-1) has read it
                if t >= 1:
                    vector.wait_ge(pool_sem, yac[t - 1])
                step(vector.reciprocal(out=R[:], in_=Pt[:].bitcast(f32)))
                # SC[t%4] free once Pool Q(t-4) has read it
                if t >= 4:
                    vector.wait_ge(pool_sem, pq[t - 4])
                step(vector.tensor_scalar(
                    out=SC[t % 4][:], in0=Pt[:].bitcast(f32),
                    scalar1=float(2.0 ** (-mbits)), scalar2=None, op0=Alu.mult,
                ))
                # ya' = x * 2^(1-emax), low half of c (Pool does the high
                # half); YA[t%2] free once u16(t-2) has read it
                if t >= 2:
                    vector.wait_ge(act_sem, u16c[t - 2])
                step(vector.tensor_tensor(
                    out=YA[t % 2][:, 0:HC, :], in0=X[t % 2][:, 0:HC, :],
                    in1=bch(R[:]), op=Alu.mult,
                ))
                assert frontd[t] == k

            def back(t):
                nonlocal k
                # V = t16 + u16 -> U16[t%2]
                vector.wait_ge(act_sem, u16c[t])
                step(vector.tensor_tensor(
                    out=U16[t % 2][:], in0=T16[t % 2][:], in1=U16[t % 2][:],
                    op=Alu.add,
                ))
                assert vd[t] == k
                # g = (t16 >> 15) & 1 -> T16 (in place; [x<0] as i16)
                step(vector.tensor_scalar(
                    out=T16[t % 2][:], in0=T16[t % 2][:],
                    scalar1=15, scalar2=1,
                    op0=Alu.logical_shift_right, op1=Alu.bitwise_and,
                ))
                # VW = V + g -> T16
                step(vector.tensor_tensor(
                    out=T16[t % 2][:], in0=U16[t % 2][:], in1=T16[t % 2][:],
                    op=Alu.add,
                ))
                assert vwd[t] == k
                # TR = VW & ~1 == 2*trunc(ya)
                step(vector.tensor_scalar(
                    out=T16[t % 2][:], in0=T16[t % 2][:],
                    scalar1=-2, scalar2=None, op0=Alu.bitwise_and,
                ))
                assert trd[t] == k

            front(0)
            front(1)
            for t in range(2, TILES):
                back(t - 2)
                front(t)
            back(TILES - 2)
            back(TILES - 1)

        def issue_stores(eng, t):
            qb = QO[t % 2]
            for b in range(B):
                eng.dma_start(
                    out=dst[t, b], in_=qb[b * SIG:(b + 1) * SIG]
                ).then_inc(store_sem, 16)

        @block.scalar
        def _(scalar):
            act_scale = float(2.0 ** (mbits - 1))
            a = 0

            def amark():
                nonlocal a
                a += 1
                return a

            for t in range(TILES):
                # t16 = rne16(2^(mb-1)*ya' - 0.5); T16[t%2] free after Q(t-2)
                if t >= 2:
                    scalar.wait_ge(pool_sem, pq[t - 2])
                scalar.wait_ge(dve_sem, frontd[t])
                scalar.wait_ge(pool_sem, yac[t])
                scalar.activation(
                    out=T16[t % 2][:], in_=YA[t % 2][:],
                    func=mybir.ActivationFunctionType.Copy,
                    bias=-0.5, scale=act_scale,
                ).then_inc(act_sem, 1)
                assert t16c[t] == amark()
                # u16 = rne16(2^(mb-1)*ya' + 0.5); U16[t%2] free after VW(t-2)
                if t >= 2:
                    scalar.wait_ge(dve_sem, vwd[t - 2])
                scalar.activation(
                    out=U16[t % 2][:], in_=YA[t % 2][:],
                    func=mybir.ActivationFunctionType.Copy,
                    bias=0.5, scale=act_scale,
                ).then_inc(act_sem, 1)
                assert u16c[t] == amark()
                # stores lag two tiles (pq[t-2] already awaited above)
                if t >= 2:
                    issue_stores(scalar, t - 2)
            scalar.wait_ge(pool_sem, pq[TILES - 2])
            issue_stores(scalar, TILES - 2)
            scalar.wait_ge(pool_sem, pq[TILES - 1])
            issue_stores(scalar, TILES - 1)

        @block.gpsimd
        def _(pool):
            p = 0

            def pmark():
                nonlocal p
                p += 1
                return p

            def yah2(t):
                # ya' high half of c; needs R(t) and loads(t); YA[t%2] free
                # once u16(t-2) has read it
                pool.wait_ge(dve_sem, rc[t])
                pool.wait_ge(load_sem, INC * (t + 1))
                if t >= 2:
                    pool.wait_ge(act_sem, u16c[t - 2])
                pool.tensor_tensor(
                    out=YA[t % 2][:, HC:C_IN, :], in0=X[t % 2][:, HC:C_IN, :],
                    in1=bch(R[:]), op=Alu.mult,
                ).then_inc(pool_sem, 1)
                assert yac[t] == pmark()

            def q(t):
                # Q = TR * bc(SC2) -> QO[t%2]; QO free after store(t-2)
                pool.wait_ge(dve_sem, trd[t])
                if t >= 2:
                    pool.wait_ge(store_sem, INC * (t - 1))
                pool.tensor_tensor(
                    out=QO[t % 2][:], in0=T16[t % 2][:], in1=bc(SC[t % 4][:]),
                    op=Alu.mult,
                ).then_inc(pool_sem, 1)
                assert pq[t] == pmark()

            yah2(0)
            for t in range(1, TILES):
                yah2(t)
                q(t - 1)
            q(TILES - 1)

        def issue_loads(sync, t):
            xb = X[t % 2]
            for b in range(B):
                sync.dma_start(
                    out=xb[b * SIG:(b + 1) * SIG], in_=src[t, b]
                ).then_inc(load_sem, 16)

        @block.sync
        def _(sync):
            issue_loads(sync, 0)
            sync.wait_ge(load_sem, INC)
            issue_loads(sync, 1)
            for t in range(TILES - 2):
                # X[t%2] free after DVE front(t) and Pool ya'-high(t)
                sync.wait_ge(dve_sem, frontd[t])
                sync.wait_ge(pool_sem, yac[t])
                issue_loads(sync, t + 2)

    return nc


def kernel(activations, mantissa_bits, blk, **_ignored):
    global LAST_RESULTS
    from concourse.bass_utils import run_bass_kernel_spmd

    mbits = int(mantissa_bits)
    assert int(blk) == C_IN, f"kernel hardcodes blk=32, got {blk}"
    x = np.ascontiguousarray(np.asarray(activations), dtype=np.float32)
    assert x.shape == (N, C, H, W), x.shape

    if mbits not in _CACHE:
        _CACHE[mbits] = _build(mbits)
    nc = _CACHE[mbits]

    shards = x.reshape(N_CORES, N_PER_CORE, C, HW)
    in_maps = [{"x": shards[i]} for i in range(N_CORES)]
    res = run_bass_kernel_spmd(nc, in_maps, list(range(N_CORES)), trace=TRACE)
    LAST_RESULTS = res
    out = np.stack([res.results[i]["q"] for i in range(N_CORES)], axis=0)
    return out.reshape(N, C, H, W)


# revision 47
# speedup vs baseline: 1.3731x; 1.3731x over previous
"""BFP activation quantization kernel for Trainium2 (8 NeuronCores).

Problem: x (64, 256, 56, 56) fp32. Channels grouped in blocks of 32; each
block shares the max frexp-exponent emax; mantissas truncated to
`mantissa_bits` bits relative to 2^emax:
    q = trunc(x / 2^(emax-mb)) * 2^(emax-mb)

Math (bit-exact):
  - emax: Pw = bits(max_block |x|) & 0x7F800000 = 2^(emax-1) (bitcast);
    R = reciprocal(Pw) = 2^(1-emax) exact (power of two).
  - ya' = x * R (exact, |ya'| < 2). ACT folds the remaining 2^(mb-1)
    factor into its per-instruction scale: with ya = x * 2^(mb-emax),
      t16 = rne16(2^(mb-1)*ya' - 0.5),  u16 = rne16(2^(mb-1)*ya' + 0.5)
    (the fp32->int16 converter rounds to nearest-even; ya +- 0.5 exact).
  - Signed trunc without an abs/sign-restore pass, exact for every case
    including rne ties and both zeros:
      2*trunc(ya) = (t16 + u16 + [x<0]) & ~1
    with [x<0] = (t16 >> 15) & 1 (t16 < 0 iff ya < 0).
  - q = (2*trunc(ya)) * 2^(emax-mb-1); int16 -> f32 convert and the
    power-of-two multiply are exact. SC2 = 2^(emax-mb-1) in bf16 (exact).

Engine split per tile (1 image; instruction cost model numbers):
  DVE  (24.2us): reduce |x| over block (strided); Pt; R; SC2; ya';
                 V = t16+u16; g = (t16>>15)&1; VW = V+g; TR = VW & ~1.
  ACT  (10.9us + store issue): t16; u16; store DMAs (lagged 2 tiles so
                 the pool wait never blocks compute dispatch).
  Pool (12.5us): Q = TR * bc(SC2)   (i16 x bf16 broadcast -> f32).
  SP:  load DMAs only (issued eagerly; never blocked behind stores).
DMA fabric floor is ~17.8us/tile (51.4 MB/core at ~360 GB/s).

Pool/ACT offload notes (probed on HW): Pool tensor_tensor supports
mult with f32 out (incl. broadcast + mixed i16/bf16 in) but NOT
max/min/abs_max or i16 out; tensor_scalar/scalar_tensor_tensor do not
exist on Pool; AluOpType.mod doesn't codegen anywhere. DVE/ACT/Pool
have no same-engine RAW interlocks -> every cross-instruction
dependency gets a standalone wait_ge on a semaphore counter; all
counter values are precomputed so engine programs can be emitted in
any order.

Layout (per core, n-shard of 8 images): tile = 1 image; SBUF partition
p = (b<8, sigma<16), free = (c<32, s<196), s contiguous in DRAM (784 B
runs).

Sharding: data-parallel on N across 8 cores, no cross-core communication.
"""

import numpy as np

N_CORES = 8
N, C, H, W = 64, 256, 56, 56
HW = H * W                   # 3136
N_PER_CORE = N // N_CORES    # 8
B = 8                        # channel blocks
SIG = 16                     # spatial chunks per image
C_IN = 32                    # channels per block
S = HW // SIG                # 196
P = B * SIG                  # 128 partitions
TILES = N_PER_CORE           # 8 (one image per tile)
DMAS = B                     # dma_starts per tile per direction
INC = 16 * DMAS              # load-sem increment per tile (128)

TRACE = False
LAST_RESULTS = None
_CACHE = {}


def _counters():
    """Precompute per-engine semaphore counter values."""
    # DVE: order f0 f1 b0 f2 b1 ... f7 b6 b7 qd7; front=4 insts, back=4,
    # plus a final low-half Q for the last tile (tail shortening).
    red, ptc, rc, frontd, vd, vwd, trd = {}, {}, {}, {}, {}, {}, {}
    k = 0
    order = [("f", 0), ("f", 1)]
    for t in range(2, TILES):
        order += [("b", t - 2), ("f", t)]
    order += [("b", TILES - 2), ("b", TILES - 1)]
    for kind, t in order:
        if kind == "f":
            red[t] = k + 1
            ptc[t] = k + 2   # Pt done
            rc[t] = k + 3    # R (reciprocal) done
            k += 4
            frontd[t] = k
        else:
            vd[t] = k + 1
            vwd[t] = k + 3
            k += 4
            trd[t] = k
    qdc = k + 1              # DVE low-half Q of last tile done

    # ACT: per t: sc2(t), t16(t), u16(t). (Store DMAs don't inc act_sem.)
    sc2c = {t: 3 * t + 1 for t in range(TILES)}
    t16c = {t: 3 * t + 2 for t in range(TILES)}
    u16c = {t: 3 * t + 3 for t in range(TILES)}

    # Pool: YAh2(0); per t>=1: YAh2(t), Q(t-1); then Qh2(TILES-1).
    yac, pq = {0: 1}, {}
    p = 1
    for t in range(1, TILES):
        yac[t] = p = p + 1
        pq[t - 1] = p = p + 1
    pq[TILES - 1] = p = p + 1

    return red, ptc, rc, frontd, vd, vwd, trd, qdc, sc2c, t16c, u16c, yac, pq


def _build(mbits: int):
    import concourse.bass as bass
    from concourse import mybir

    nc = bass.Bass()
    x_in = nc.declare_dram_parameter(
        "x", [N_PER_CORE, C, HW], mybir.dt.float32, isOutput=False
    )
    q_out = nc.declare_dram_parameter(
        "q", [N_PER_CORE, C, HW], mybir.dt.float32, isOutput=True
    )
    src = x_in[:].rearrange("n (b c) (g s) -> n b g c s", c=C_IN, s=S)
    dst = q_out[:].rearrange("n (b c) (g s) -> n b g c s", c=C_IN, s=S)

    i32, f32, i16, bf16 = (
        mybir.dt.int32, mybir.dt.float32, mybir.dt.int16, mybir.dt.bfloat16
    )
    Alu = mybir.AluOpType

    (red, ptc, rc, frontd, vd, vwd, trd, qdc,
     sc2c, t16c, u16c, yac, pq) = _counters()

    from contextlib import ExitStack
    es = ExitStack()
    with es:
        sb = lambda nm, shape, dt: es.enter_context(nc.sbuf_tensor(nm, shape, dt))
        X0 = sb("X0", [P, C_IN, S], f32); X1 = sb("X1", [P, C_IN, S], f32)
        YA0 = sb("YA0", [P, C_IN, S], f32); YA1 = sb("YA1", [P, C_IN, S], f32)
        Q0 = sb("Q0", [P, C_IN, S], f32); Q1 = sb("Q1", [P, C_IN, S], f32)
        T0 = sb("T0", [P, C_IN, S], i16); T1 = sb("T1", [P, C_IN, S], i16)
        U0 = sb("U0", [P, C_IN, S], i16); U1 = sb("U1", [P, C_IN, S], i16)
        M = sb("Mt", [P, S], f32); Pt = sb("Ptt", [P, S], i32)
        R = sb("Rt", [P, S], f32)
        SCS = [sb(f"SC{i}", [P, S], bf16) for i in range(4)]
        load_sem = es.enter_context(nc.semaphore())
        store_sem = es.enter_context(nc.semaphore())
        dve_sem = es.enter_context(nc.semaphore())
        act_sem = es.enter_context(nc.semaphore())
        pool_sem = es.enter_context(nc.semaphore())
        block = es.enter_context(nc.Block())
        X = [X0, X1]
        YA = [YA0, YA1]
        QO = [Q0, Q1]
        T16 = [T0, T1]
        U16 = [U0, U1]
        SC = SCS
        HC = C_IN // 2       # low/high halves for the last tile's Q split
        YD = 14              # ya' split: DVE does c<14, Pool does c>=14
                             # (DVE is the bottleneck; Pool is 1.9x slower
                             # per element, balance at 14/18)

        def bcw(ap, w):
            return ap.unsqueeze(1).broadcast_to((P, w, S))

        def bch(ap):
            return bcw(ap, HC)

        def bc(ap):
            return ap.unsqueeze(1).broadcast_to((P, C_IN, S))

        @block.vector
        def _(vector):
            k = 0

            def step(inst):
                nonlocal k
                inst.then_inc(dve_sem, 1)
                k += 1
                vector.wait_ge(dve_sem, k)

            def front(t):
                nonlocal k
                # M = max_block |x| (strided free-dim reduce)
                vector.wait_ge(load_sem, INC * (t + 1))
                step(vector.tensor_reduce(
                    out=M[:], in_=X[t % 2][:].rearrange("p c s -> p s c"),
                    axis=mybir.AxisListType.X, op=Alu.max,
                    apply_absolute_value=True,
                ))
                assert red[t] == k
                # Pt free once ACT sc2(t-1) has read it
                if t >= 1:
                    vector.wait_ge(act_sem, sc2c[t - 1])
                step(vector.tensor_scalar(
                    out=Pt[:], in0=M[:].bitcast(i32),
                    scalar1=0x7F800000, scalar2=None, op0=Alu.bitwise_and,
                ))
                assert ptc[t] == k
                # R free once Pool ya'-high(t-1) has read it
                if t >= 1:
                    vector.wait_ge(pool_sem, yac[t - 1])
                step(vector.reciprocal(out=R[:], in_=Pt[:].bitcast(f32)))
                # ya' = x * 2^(1-emax), low half of c (Pool does the high
                # half); YA[t%2] free once u16(t-2) has read it
                if t >= 2:
                    vector.wait_ge(act_sem, u16c[t - 2])
                step(vector.tensor_tensor(
                    out=YA[t % 2][:, 0:YD, :], in0=X[t % 2][:, 0:YD, :],
                    in1=bcw(R[:], YD), op=Alu.mult,
                ))
                assert frontd[t] == k

            def back(t):
                nonlocal k
                # V = t16 + u16 -> U16[t%2]
                vector.wait_ge(act_sem, u16c[t])
                step(vector.tensor_tensor(
                    out=U16[t % 2][:], in0=T16[t % 2][:], in1=U16[t % 2][:],
                    op=Alu.add,
                ))
                assert vd[t] == k
                # g = (t16 >> 15) & 1 -> T16 (in place; [x<0] as i16)
                step(vector.tensor_scalar(
                    out=T16[t % 2][:], in0=T16[t % 2][:],
                    scalar1=15, scalar2=1,
                    op0=Alu.logical_shift_right, op1=Alu.bitwise_and,
                ))
                # VW = V + g -> T16
                step(vector.tensor_tensor(
                    out=T16[t % 2][:], in0=U16[t % 2][:], in1=T16[t % 2][:],
                    op=Alu.add,
                ))
                assert vwd[t] == k
                # TR = VW & ~1 == 2*trunc(ya)
                step(vector.tensor_scalar(
                    out=T16[t % 2][:], in0=T16[t % 2][:],
                    scalar1=-2, scalar2=None, op0=Alu.bitwise_and,
                ))
                assert trd[t] == k

            front(0)
            front(1)
            for t in range(2, TILES):
                back(t - 2)
                front(t)
            back(TILES - 2)
            back(TILES - 1)
            # tail: low half of the last tile's Q on the (now idle) DVE
            tl = TILES - 1
            vector.wait_ge(store_sem, INC * (tl - 1))
            step(vector.tensor_tensor(
                out=QO[tl % 2][:, 0:HC, :], in0=T16[tl % 2][:, 0:HC, :],
                in1=bch(SC[tl % 4][:]), op=Alu.mult,
            ))
            assert qdc == k

        def issue_stores(eng, t):
            qb = QO[t % 2]
            for b in range(B):
                eng.dma_start(
                    out=dst[t, b], in_=qb[b * SIG:(b + 1) * SIG]
                ).then_inc(store_sem, 16)

        @block.scalar
        def _(scalar):
            act_scale = float(2.0 ** (mbits - 1))
            a = 0

            def amark():
                nonlocal a
                a += 1
                return a

            for t in range(TILES):
                # SC2 = 2^(emax-mb-1) as bf16; SC[t%4] free after Q(t-4)
                if t >= 4:
                    scalar.wait_ge(pool_sem, pq[t - 4])
                scalar.wait_ge(dve_sem, ptc[t])
                scalar.activation(
                    out=SC[t % 4][:], in_=Pt[:].bitcast(f32),
                    func=mybir.ActivationFunctionType.Copy,
                    bias=0.0, scale=float(2.0 ** (-mbits)),
                ).then_inc(act_sem, 1)
                assert sc2c[t] == amark()
                # t16 = rne16(2^(mb-1)*ya' - 0.5); T16[t%2] free after Q(t-2)
                if t >= 2:
                    scalar.wait_ge(pool_sem, pq[t - 2])
                scalar.wait_ge(dve_sem, frontd[t])
                scalar.wait_ge(pool_sem, yac[t])
                scalar.activation(
                    out=T16[t % 2][:], in_=YA[t % 2][:],
                    func=mybir.ActivationFunctionType.Copy,
                    bias=-0.5, scale=act_scale,
                ).then_inc(act_sem, 1)
                assert t16c[t] == amark()
                # u16 = rne16(2^(mb-1)*ya' + 0.5); U16[t%2] free after VW(t-2)
                if t >= 2:
                    scalar.wait_ge(dve_sem, vwd[t - 2])
                scalar.activation(
                    out=U16[t % 2][:], in_=YA[t % 2][:],
                    func=mybir.ActivationFunctionType.Copy,
                    bias=0.5, scale=act_scale,
                ).then_inc(act_sem, 1)
                assert u16c[t] == amark()
                # stores lag two tiles (pq[t-2] already awaited above)
                if t >= 2:
                    issue_stores(scalar, t - 2)
            scalar.wait_ge(pool_sem, pq[TILES - 2])
            issue_stores(scalar, TILES - 2)
            # last tile: ACT issues blocks 0..3, SP issues 4..7 in parallel
            scalar.wait_ge(pool_sem, pq[TILES - 1])
            scalar.wait_ge(dve_sem, qdc)
            tl = TILES - 1
            for b in range(B // 2):
                scalar.dma_start(
                    out=dst[tl, b],
                    in_=QO[tl % 2][b * SIG:(b + 1) * SIG],
                ).then_inc(store_sem, 16)

        @block.gpsimd
        def _(pool):
            p = 0

            def pmark():
                nonlocal p
                p += 1
                return p

            def yah2(t):
                # ya' high half of c; needs R(t) and loads(t); YA[t%2] free
                # once u16(t-2) has read it
                pool.wait_ge(dve_sem, rc[t])
                pool.wait_ge(load_sem, INC * (t + 1))
                if t >= 2:
                    pool.wait_ge(act_sem, u16c[t - 2])
                pool.tensor_tensor(
                    out=YA[t % 2][:, YD:C_IN, :], in0=X[t % 2][:, YD:C_IN, :],
                    in1=bcw(R[:], C_IN - YD), op=Alu.mult,
                ).then_inc(pool_sem, 1)
                assert yac[t] == pmark()

            def q(t):
                # Q = TR * bc(SC2) -> QO[t%2]; QO free after store(t-2)
                # (for the last tile DVE covers the low half of c)
                lo = HC if t == TILES - 1 else 0
                pool.wait_ge(dve_sem, trd[t])
                if t >= 2:
                    pool.wait_ge(store_sem, INC * (t - 1))
                pool.tensor_tensor(
                    out=QO[t % 2][:, lo:C_IN, :],
                    in0=T16[t % 2][:, lo:C_IN, :],
                    in1=(bc if lo == 0 else bch)(SC[t % 4][:]),
                    op=Alu.mult,
                ).then_inc(pool_sem, 1)
                assert pq[t] == pmark()

            yah2(0)
            for t in range(1, TILES):
                yah2(t)
                q(t - 1)
            q(TILES - 1)

        def issue_loads(sync, t):
            xb = X[t % 2]
            for b in range(B):
                sync.dma_start(
                    out=xb[b * SIG:(b + 1) * SIG], in_=src[t, b]
                ).then_inc(load_sem, 16)

        @block.sync
        def _(sync):
            issue_loads(sync, 0)
            sync.wait_ge(load_sem, INC)
            issue_loads(sync, 1)
            for t in range(TILES - 2):
                # X[t%2] free after DVE front(t) and Pool ya'-high(t)
                sync.wait_ge(dve_sem, frontd[t])
                sync.wait_ge(pool_sem, yac[t])
                issue_loads(sync, t + 2)
            # last tile: SP issues store blocks 4..7 (ACT does 0..3)
            tl = TILES - 1
            sync.wait_ge(pool_sem, pq[tl])
            sync.wait_ge(dve_sem, qdc)
            for b in range(B // 2, B):
                sync.dma_start(
                    out=dst[tl, b],
                    in_=QO[tl % 2][b * SIG:(b + 1) * SIG],
                ).then_inc(store_sem, 16)

    return nc


def kernel(activations, mantissa_bits, blk, **_ignored):
    global LAST_RESULTS
    from concourse.bass_utils import run_bass_kernel_spmd

    mbits = int(mantissa_bits)
    assert int(blk) == C_IN, f"kernel hardcodes blk=32, got {blk}"
    x = np.ascontiguousarray(np.asarray(activations), dtype=np.float32)
    assert x.shape == (N, C, H, W), x.shape

    if mbits not in _CACHE:
        _CACHE[mbits] = _build(mbits)
    nc = _CACHE[mbits]

    shards = x.reshape(N_CORES, N_PER_CORE, C, HW)
    in_maps = [{"x": shards[i]} for i in range(N_CORES)]
    res = run_bass_kernel_spmd(nc, in_maps, list(range(N_CORES)), trace=TRACE)
    LAST_RESULTS = res
    out = np.stack([res.results[i]["q"] for i in range(N_CORES)], axis=0)
    return out.reshape(N, C, H, W)
